# revision 6
# baseline (speedup 1.0000x reference)
"""Block-diagonal dense (nn_BlockDiagonalDense) Trainium2 Bass kernel.

Math: x [B=4, T=4096, F=4096] fp32; per token, features are grouped into
512 blocks of 8; each block is multiplied by its own 8x8 matrix
(kernel [16 heads, 32 blocks, 8, 8]) and bias added.

Strategy (v2, fp16 weight-stationary, transposed I/O):
  - Data-parallel over tokens across 8 cores (16384 tokens -> 2048/core).
  - The op is memory-bound (1 GFLOP vs 512 MiB fp32 traffic), and the
    rel-err budget (2e-2) dwarfs fp16 rounding (~1e-3), so all device I/O
    is fp16: half the HBM bytes of the fp32 baseline.
  - Host pre-transposes each core's token slice to x^T [4096 feat, 2048 tok]
    fp16. This removes the on-chip PE transpose entirely: features land on
    partitions, so the per-128-feature-chunk block-diagonal matmul runs
    weight-stationary (lhsT = W chunk [fin,fout], rhs = x^T chunk moving,
    out = y^T [fout, tok] in PSUM).
  - Weights are expanded host-side into 32 chunks of 128x128 block-diagonal
    matrices (fp16), all resident in SBUF; bias folded into the PSUM->SBUF
    drain (DVE tensor_scalar_add on one half, ACT Identity-add on the other,
    so neither engine becomes the bottleneck).
  - Device writes y^T fp16 contiguously; host transposes back and upcasts.
  - Per chunk: 512 KiB in (SP ring), 4 matmuls ap=512, 2 drains, 512 KiB
    out (ACT ring). 32 chunks/core, fully pipelined via tile pools.
"""

import sys

if "/opt/trn_rl_repo" not in sys.path:
    sys.path.insert(0, "/opt/trn_rl_repo")

import numpy as np

NUM_HEADS = 16
BLOCK_SIZE = 8
FEATURES = 4096
HEAD_DIM = FEATURES // NUM_HEADS  # 256
BLOCK_DIM = HEAD_DIM // BLOCK_SIZE  # 32

N_CORES = 8
TOKENS_TOTAL = 4 * 4096  # 16384
TOK_PER_CORE = TOKENS_TOTAL // N_CORES  # 2048

P = 128  # partitions
N_CHUNKS = FEATURES // P  # 32 chunks of 128 features

_NC_CACHE = {}


def build_nc(
    tok_per_core=TOK_PER_CORE,
    repeats=1,
    dma_pattern="split",
    drain_split=1024,
    xbufs=6,
    ybufs=6,
    psbufs=2,
):
    """Build the Bass program for one core processing x^T [4096, tok_per_core].

    repeats>1 wraps the whole body in a hardware loop doing identical work
    (same inputs, same outputs) -- used only for slope-based device timing.

    dma_pattern: "split" = x^T in on SP ring / y^T out on ACT ring;
                 "alt2"  = both rings alternate directions per chunk;
                 "alt3"  = SP + ACT + SWDGE(gpsimd) rotate.
    """
    import contextlib

    import concourse.mybir as mybir
    from concourse import bacc
    from concourse.tile import TileContext

    f32 = mybir.dt.float32
    f16 = mybir.dt.float16
    nc = bacc.Bacc(None, target_bir_lowering=False)

    T = tok_per_core
    xt = nc.declare_dram_parameter("xt", [FEATURES, T], f16, isOutput=False)
    # w: [128 (fi within chunk), 32*128 (chunk-major, fo within chunk)]
    w = nc.declare_dram_parameter("w", [P, N_CHUNKS * P], f16, isOutput=False)
    # b: [128 (fo within chunk), 32 (chunk)]
    b = nc.declare_dram_parameter("b", [P, N_CHUNKS], f32, isOutput=False)
    yt = nc.declare_dram_parameter("yt", [FEATURES, T], f16, isOutput=True)

    TS = T // 4  # one PSUM bank's worth of tokens (512 fp32)

    with TileContext(nc) as tc:
        with (
            tc.tile_pool(name="const", bufs=1) as const_pool,
            tc.tile_pool(name="xin", bufs=xbufs) as x_pool,
            tc.tile_pool(name="yout", bufs=ybufs) as y_pool,
            tc.tile_pool(name="ps", bufs=psbufs, space="PSUM") as ps_pool,
        ):
            w_sb = const_pool.tile([P, N_CHUNKS * P], f16)
            nc.scalar.dma_start(out=w_sb, in_=w[:, :])
            b_sb = const_pool.tile([P, N_CHUNKS], f32)
            nc.scalar.dma_start(out=b_sb, in_=b[:, :])

            if dma_pattern == "split":
                in_engines, out_engines = (nc.sync,), (nc.scalar,)
            elif dma_pattern == "alt2":
                in_engines, out_engines = (nc.sync, nc.scalar), (nc.scalar, nc.sync)
            elif dma_pattern == "alt3":
                in_engines = (nc.sync, nc.gpsimd, nc.scalar)
                out_engines = (nc.scalar, nc.sync, nc.gpsimd)
            else:
                raise ValueError(dma_pattern)

            rep_ctx = (
                tc.For_i(0, repeats, 1) if repeats > 1 else contextlib.nullcontext()
            )
            with rep_ctx:
                for c in range(N_CHUNKS):
                    rows = slice(c * P, (c + 1) * P)
                    xt_sb = x_pool.tile([P, T], f16)
                    in_engines[c % len(in_engines)].dma_start(
                        out=xt_sb, in_=xt[rows, :]
                    )

                    ps = ps_pool.tile([P, T], f32)
                    for t in range(4):
                        nc.tensor.matmul(
                            ps[:, t * TS : (t + 1) * TS],
                            w_sb[:, c * P : (c + 1) * P],
                            xt_sb[:, t * TS : (t + 1) * TS],
                        )

                    y_sb = y_pool.tile([P, T], f16)
                    bc = b_sb[:, c : c + 1]
                    # split the drain across DVE and ACT so neither is the
                    # bottleneck; both fold in the bias add + fp32->fp16 cast
                    nc.vector.tensor_scalar_add(
                        y_sb[:, :drain_split], ps[:, :drain_split], bc
                    )
                    nc.scalar.add(y_sb[:, drain_split:], ps[:, drain_split:], bc)

                    out_engines[c % len(out_engines)].dma_start(
                        out=yt[rows, :], in_=y_sb
                    )

    nc.finalize()
    return nc


def expand_weights(kern):
    """kernel [16, 32, 8, 8] -> [128, 32*128] chunk-major block-diagonal."""
    kern = np.asarray(kern, dtype=np.float32)
    wd = np.zeros((N_CHUNKS, P, P), dtype=np.float32)
    for c in range(N_CHUNKS):
        h = c // 2
        for j in range(16):
            bd = 16 * (c % 2) + j
            wd[c, 8 * j : 8 * j + 8, 8 * j : 8 * j + 8] = kern[h, bd]
    # [chunk, fi, fo] -> [fi, chunk*128 + fo]
    return np.ascontiguousarray(wd.transpose(1, 0, 2).reshape(P, N_CHUNKS * P))


def make_in_maps(x, kern, bias):
    """Host-side prep: shard + transpose + downcast. Returns per-core dicts."""
    xf = np.asarray(x, dtype=np.float32).reshape(TOKENS_TOTAL, FEATURES)
    w16 = expand_weights(kern).astype(np.float16)
    bmat = np.ascontiguousarray(
        np.asarray(bias, dtype=np.float32).reshape(N_CHUNKS, P).T
    )
    maps = []
    for c in range(N_CORES):
        sl = xf[c * TOK_PER_CORE : (c + 1) * TOK_PER_CORE]
        xt_c = np.ascontiguousarray(sl.astype(np.float16).T)
        maps.append({"xt": xt_c, "w": w16, "b": bmat})
    return maps


def reference_numpy(x, kern, bias):
    xb = np.asarray(x, np.float32).reshape(-1, NUM_HEADS, BLOCK_DIM, BLOCK_SIZE)
    k = np.asarray(kern, np.float32)
    y = np.einsum("nhbs,hbst->nhbt", xb, k) + np.asarray(bias, np.float32)
    return y.reshape(x.shape)


_LAST_EXEC_NS = None


def _build_runner(nc):
    """One-time compiled executor for repeat calls: a single cached jit
    (vs run_bass_via_pjrt's per-call closure, which re-compiles and leaks
    ~0.4GB RSS per call) plus device-resident output-zero buffers."""
    import jax
    from jax.experimental.shard_map import shard_map
    from jax.sharding import Mesh, NamedSharding, PartitionSpec

    import concourse.mybir as mybir
    from concourse import bass2jax

    bass2jax.install_neuronx_cc_hook()
    partition_name = nc.partition_id_tensor.name if nc.partition_id_tensor else None
    in_names, out_names, out_avals, zeros = [], [], [], []
    for alloc in nc.m.functions[0].allocations:
        if not isinstance(alloc, mybir.MemoryLocationSet):
            continue
        name = alloc.memorylocations[0].name
        if alloc.kind == "ExternalInput":
            if name != partition_name:
                in_names.append(name)
        elif alloc.kind == "ExternalOutput":
            shape = tuple(alloc.tensor_shape)
            dt = mybir.dt.np(alloc.dtype)
            out_names.append(name)
            out_avals.append(jax.core.ShapedArray(shape, dt))
            zeros.append(np.zeros(shape, dt))
    all_names = in_names + out_names + ([partition_name] if partition_name else [])

    def _body(*args):
        ops = list(args)
        if partition_name is not None:
            ops.append(bass2jax.partition_id_tensor())
        return tuple(
            bass2jax._bass_exec_p.bind(
                *ops,
                out_avals=tuple(out_avals),
                in_names=tuple(all_names),
                out_names=tuple(out_names),
                lowering_input_output_aliases=(),
                sim_require_finite=True,
                sim_require_nnan=True,
                nc=nc,
            )
        )

    devices = jax.devices()[:N_CORES]
    mesh = Mesh(np.asarray(devices), ("core",))
    n_all = len(in_names) + len(out_names)
    fn = jax.jit(
        shard_map(
            _body,
            mesh=mesh,
            in_specs=(PartitionSpec("core"),) * n_all,
            out_specs=(PartitionSpec("core"),) * len(out_names),
            check_rep=False,
        ),
        keep_unused=True,
    )
    shard = NamedSharding(mesh, PartitionSpec("core"))
    dev_zeros = [
        jax.device_put(np.zeros((N_CORES * z.shape[0], *z.shape[1:]), z.dtype), shard)
        for z in zeros
    ]
    out_shapes = [z.shape for z in zeros]
    return fn, in_names, out_names, out_shapes, dev_zeros, shard


def _run_cached(nc, in_maps):
    import jax

    if "runner" not in _NC_CACHE:
        _NC_CACHE["runner"] = _build_runner(nc)
    fn, in_names, out_names, out_shapes, dev_zeros, shard = _NC_CACHE["runner"]
    concat_in = [
        np.concatenate([np.asarray(m[n]) for m in in_maps], axis=0)
        for n in in_names
    ]
    dev_in = [jax.device_put(a, shard) for a in concat_in]
    outs = fn(*dev_in, *dev_zeros)
    return [
        {
            name: np.asarray(outs[i]).reshape(N_CORES, *out_shapes[i])[c]
            for i, name in enumerate(out_names)
        }
        for c in range(N_CORES)
    ]


def _assemble(results, orig_shape, bias):
    y = np.concatenate([r["yt"].T.astype(np.float32) for r in results], axis=0)
    return y.reshape(orig_shape)


def _sample_ok(y, x2d, kern, bias_flat):
    """Exact numpy check on a 512-token sample (~30 MFLOP)."""
    rows = np.arange(0, TOKENS_TOTAL, TOKENS_TOTAL // 512)[:512]
    xb = x2d[rows].reshape(len(rows), NUM_HEADS, BLOCK_DIM, BLOCK_SIZE)
    yref = (
        np.einsum("nhbs,hbst->nhbt", xb, np.asarray(kern, np.float32)).reshape(
            len(rows), FEATURES
        )
        + bias_flat
    )
    ys = y.reshape(TOKENS_TOTAL, FEATURES)[rows]
    err = np.linalg.norm(ys - yref) / max(float(np.linalg.norm(yref)), 1e-30)
    return err < 5e-3


def kernel(**inputs):
    """Full inputs in, full output out. Shards tokens across 8 cores."""
    global _LAST_EXEC_NS
    import os

    from concourse.bass_utils import run_bass_kernel_spmd

    x = np.asarray(inputs["x"], dtype=np.float32)
    orig_shape = x.shape
    kern = np.asarray(inputs["kernel"], dtype=np.float32)
    bias_flat = np.asarray(inputs["bias"], dtype=np.float32).reshape(FEATURES)

    in_maps = make_in_maps(x, kern, bias_flat)

    if "nc" not in _NC_CACHE:
        _NC_CACHE["nc"] = build_nc()
    nc = _NC_CACHE["nc"]

    trace = bool(os.environ.get("BASS_KERNEL_TRACE"))

    def _canonical():
        # honor trace requests where the NTFF hook exists, but degrade to an
        # untraced run if tracing is unavailable (missing antenv.axon_hooks);
        # the extra untraced attempt also clears transient device errors
        # (e.g. NRT_EXEC_UNIT_UNRECOVERABLE on first touch of the cores)
        global _LAST_EXEC_NS
        attempts = ([True] if trace else []) + [False, False]
        last_err = None
        for attempt_trace in attempts:
            try:
                res = run_bass_kernel_spmd(
                    nc, in_maps, list(range(N_CORES)), trace=attempt_trace
                )
                _LAST_EXEC_NS = res.exec_time_ns
                return res.results
            except Exception as e:
                last_err = e
        raise last_err

    # first call (and any traced call) uses the canonical library path;
    # repeat calls reuse one cached executable to keep wall time and RSS flat
    results = None
    if _NC_CACHE.get("calls", 0) > 0 and not trace:
        try:
            results = _run_cached(nc, in_maps)
        except Exception:
            results = None
    if results is None:
        results = _canonical()
    _NC_CACHE["calls"] = _NC_CACHE.get("calls", 0) + 1

    x2d = x.reshape(TOKENS_TOTAL, FEATURES)
    y = _assemble(results, orig_shape, bias_flat)
    if not _sample_ok(y, x2d, kern, bias_flat):
        # defense in depth: recompute via the canonical path if the sampled
        # numeric check fails for any reason
        results = _canonical()
        y = _assemble(results, orig_shape, bias_flat)
    return y.reshape(orig_shape)


# revision 7
# speedup vs baseline: 1.0007x; 1.0007x over previous
"""Block-diagonal dense (nn_BlockDiagonalDense) Trainium2 Bass kernel.

Math: x [B=4, T=4096, F=4096] fp32; per token, features are grouped into
512 blocks of 8; each block is multiplied by its own 8x8 matrix
(kernel [16 heads, 32 blocks, 8, 8]) and bias added.

Strategy (v2, fp16 weight-stationary, transposed I/O):
  - Data-parallel over tokens across 8 cores (16384 tokens -> 2048/core).
  - The op is memory-bound (1 GFLOP vs 512 MiB fp32 traffic), and the
    rel-err budget (2e-2) dwarfs fp16 rounding (~1e-3), so all device I/O
    is fp16: half the HBM bytes of the fp32 baseline.
  - Host pre-transposes each core's token slice to x^T [4096 feat, 2048 tok]
    fp16. This removes the on-chip PE transpose entirely: features land on
    partitions, so the per-128-feature-chunk block-diagonal matmul runs
    weight-stationary (lhsT = W chunk [fin,fout], rhs = x^T chunk moving,
    out = y^T [fout, tok] in PSUM).
  - Weights are expanded host-side into 32 chunks of 128x128 block-diagonal
    matrices (fp16), all resident in SBUF; bias folded into the PSUM->SBUF
    drain (DVE tensor_scalar_add on one half, ACT Identity-add on the other,
    so neither engine becomes the bottleneck).
  - Device writes y^T fp16 contiguously; host transposes back and upcasts.
  - Per chunk: 512 KiB in (SP ring), 4 matmuls ap=512, 2 drains, 512 KiB
    out (ACT ring). 32 chunks/core, fully pipelined via tile pools.
"""

import sys

if "/opt/trn_rl_repo" not in sys.path:
    sys.path.insert(0, "/opt/trn_rl_repo")

import numpy as np

NUM_HEADS = 16
BLOCK_SIZE = 8
FEATURES = 4096
HEAD_DIM = FEATURES // NUM_HEADS  # 256
BLOCK_DIM = HEAD_DIM // BLOCK_SIZE  # 32

N_CORES = 8
TOKENS_TOTAL = 4 * 4096  # 16384
TOK_PER_CORE = TOKENS_TOTAL // N_CORES  # 2048

P = 128  # partitions
N_CHUNKS = FEATURES // P  # 32 chunks of 128 features

_NC_CACHE = {}


def build_nc(
    tok_per_core=TOK_PER_CORE,
    repeats=1,
    dma_pattern="split",
    drain_split=1024,
    xbufs=6,
    ybufs=6,
    psbufs=2,
):
    """Build the Bass program for one core processing x^T [4096, tok_per_core].

    repeats>1 wraps the whole body in a hardware loop doing identical work
    (same inputs, same outputs) -- used only for slope-based device timing.

    dma_pattern: "split" = x^T in on SP ring / y^T out on ACT ring;
                 "alt2"  = both rings alternate directions per chunk;
                 "alt3"  = SP + ACT + SWDGE(gpsimd) rotate.
    """
    import contextlib

    import concourse.mybir as mybir
    from concourse import bacc
    from concourse.tile import TileContext

    f32 = mybir.dt.float32
    f16 = mybir.dt.float16
    nc = bacc.Bacc(None, target_bir_lowering=False)

    T = tok_per_core
    xt = nc.declare_dram_parameter("xt", [FEATURES, T], f16, isOutput=False)
    # w: [128 (fi within chunk), 32*128 (chunk-major, fo within chunk)]
    w = nc.declare_dram_parameter("w", [P, N_CHUNKS * P], f16, isOutput=False)
    # b: [128 (fo within chunk), 32 (chunk)]
    b = nc.declare_dram_parameter("b", [P, N_CHUNKS], f32, isOutput=False)
    yt = nc.declare_dram_parameter("yt", [FEATURES, T], f16, isOutput=True)

    TS = T // 4  # one PSUM bank's worth of tokens (512 fp32)

    with TileContext(nc) as tc:
        with (
            tc.tile_pool(name="const", bufs=1) as const_pool,
            tc.tile_pool(name="xin", bufs=xbufs) as x_pool,
            tc.tile_pool(name="yout", bufs=ybufs) as y_pool,
            tc.tile_pool(name="ps", bufs=psbufs, space="PSUM") as ps_pool,
        ):
            w_sb = const_pool.tile([P, N_CHUNKS * P], f16)
            nc.scalar.dma_start(out=w_sb, in_=w[:, :])
            b_sb = const_pool.tile([P, N_CHUNKS], f32)
            nc.scalar.dma_start(out=b_sb, in_=b[:, :])

            if dma_pattern == "split":
                in_engines, out_engines = (nc.sync,), (nc.scalar,)
            elif dma_pattern == "alt2":
                in_engines, out_engines = (nc.sync, nc.scalar), (nc.scalar, nc.sync)
            elif dma_pattern == "alt3":
                in_engines = (nc.sync, nc.gpsimd, nc.scalar)
                out_engines = (nc.scalar, nc.sync, nc.gpsimd)
            else:
                raise ValueError(dma_pattern)

            rep_ctx = (
                tc.For_i(0, repeats, 1) if repeats > 1 else contextlib.nullcontext()
            )
            with rep_ctx:
                for c in range(N_CHUNKS):
                    rows = slice(c * P, (c + 1) * P)
                    xt_sb = x_pool.tile([P, T], f16)
                    in_engines[c % len(in_engines)].dma_start(
                        out=xt_sb, in_=xt[rows, :]
                    )

                    ps = ps_pool.tile([P, T], f32)
                    for t in range(4):
                        nc.tensor.matmul(
                            ps[:, t * TS : (t + 1) * TS],
                            w_sb[:, c * P : (c + 1) * P],
                            xt_sb[:, t * TS : (t + 1) * TS],
                        )

                    y_sb = y_pool.tile([P, T], f16)
                    bc = b_sb[:, c : c + 1]
                    # split the drain across DVE and ACT so neither is the
                    # bottleneck; both fold in the bias add + fp32->fp16 cast
                    nc.vector.tensor_scalar_add(
                        y_sb[:, :drain_split], ps[:, :drain_split], bc
                    )
                    nc.scalar.add(y_sb[:, drain_split:], ps[:, drain_split:], bc)

                    out_engines[c % len(out_engines)].dma_start(
                        out=yt[rows, :], in_=y_sb
                    )

    nc.finalize()
    return nc


def expand_weights(kern):
    """kernel [16, 32, 8, 8] -> [128, 32*128] chunk-major block-diagonal."""
    kern = np.asarray(kern, dtype=np.float32)
    wd = np.zeros((N_CHUNKS, P, P), dtype=np.float32)
    for c in range(N_CHUNKS):
        h = c // 2
        for j in range(16):
            bd = 16 * (c % 2) + j
            wd[c, 8 * j : 8 * j + 8, 8 * j : 8 * j + 8] = kern[h, bd]
    # [chunk, fi, fo] -> [fi, chunk*128 + fo]
    return np.ascontiguousarray(wd.transpose(1, 0, 2).reshape(P, N_CHUNKS * P))


def make_in_maps(x, kern, bias):
    """Host-side prep: shard + transpose + downcast. Returns per-core dicts."""
    xf = np.asarray(x, dtype=np.float32).reshape(TOKENS_TOTAL, FEATURES)
    w16 = expand_weights(kern).astype(np.float16)
    bmat = np.ascontiguousarray(
        np.asarray(bias, dtype=np.float32).reshape(N_CHUNKS, P).T
    )
    maps = []
    for c in range(N_CORES):
        sl = xf[c * TOK_PER_CORE : (c + 1) * TOK_PER_CORE]
        xt_c = np.ascontiguousarray(sl.astype(np.float16).T)
        maps.append({"xt": xt_c, "w": w16, "b": bmat})
    return maps


def reference_numpy(x, kern, bias):
    xb = np.asarray(x, np.float32).reshape(-1, NUM_HEADS, BLOCK_DIM, BLOCK_SIZE)
    k = np.asarray(kern, np.float32)
    y = np.einsum("nhbs,hbst->nhbt", xb, k) + np.asarray(bias, np.float32)
    return y.reshape(x.shape)


_LAST_EXEC_NS = None


def _build_runner(nc):
    """One-time compiled executor for repeat calls: a single cached jit
    (vs run_bass_via_pjrt's per-call closure, which re-compiles and leaks
    ~0.4GB RSS per call) plus device-resident output-zero buffers."""
    import jax
    from jax.experimental.shard_map import shard_map
    from jax.sharding import Mesh, NamedSharding, PartitionSpec

    import concourse.mybir as mybir
    from concourse import bass2jax

    bass2jax.install_neuronx_cc_hook()
    partition_name = nc.partition_id_tensor.name if nc.partition_id_tensor else None
    in_names, out_names, out_avals, zeros = [], [], [], []
    for alloc in nc.m.functions[0].allocations:
        if not isinstance(alloc, mybir.MemoryLocationSet):
            continue
        name = alloc.memorylocations[0].name
        if alloc.kind == "ExternalInput":
            if name != partition_name:
                in_names.append(name)
        elif alloc.kind == "ExternalOutput":
            shape = tuple(alloc.tensor_shape)
            dt = mybir.dt.np(alloc.dtype)
            out_names.append(name)
            out_avals.append(jax.core.ShapedArray(shape, dt))
            zeros.append(np.zeros(shape, dt))
    all_names = in_names + out_names + ([partition_name] if partition_name else [])

    def _body(*args):
        ops = list(args)
        if partition_name is not None:
            ops.append(bass2jax.partition_id_tensor())
        return tuple(
            bass2jax._bass_exec_p.bind(
                *ops,
                out_avals=tuple(out_avals),
                in_names=tuple(all_names),
                out_names=tuple(out_names),
                lowering_input_output_aliases=(),
                sim_require_finite=True,
                sim_require_nnan=True,
                nc=nc,
            )
        )

    devices = jax.devices()[:N_CORES]
    mesh = Mesh(np.asarray(devices), ("core",))
    n_all = len(in_names) + len(out_names)
    fn = jax.jit(
        shard_map(
            _body,
            mesh=mesh,
            in_specs=(PartitionSpec("core"),) * n_all,
            out_specs=(PartitionSpec("core"),) * len(out_names),
            check_rep=False,
        ),
        keep_unused=True,
    )
    shard = NamedSharding(mesh, PartitionSpec("core"))
    dev_zeros = [
        jax.device_put(np.zeros((N_CORES * z.shape[0], *z.shape[1:]), z.dtype), shard)
        for z in zeros
    ]
    out_shapes = [z.shape for z in zeros]
    return fn, in_names, out_names, out_shapes, dev_zeros, shard


def _run_cached(nc, in_maps):
    import jax

    if "runner" not in _NC_CACHE:
        _NC_CACHE["runner"] = _build_runner(nc)
    fn, in_names, out_names, out_shapes, dev_zeros, shard = _NC_CACHE["runner"]
    concat_in = [
        np.concatenate([np.asarray(m[n]) for m in in_maps], axis=0)
        for n in in_names
    ]
    dev_in = [jax.device_put(a, shard) for a in concat_in]
    outs = fn(*dev_in, *dev_zeros)
    return [
        {
            name: np.asarray(outs[i]).reshape(N_CORES, *out_shapes[i])[c]
            for i, name in enumerate(out_names)
        }
        for c in range(N_CORES)
    ]


def _assemble(results, orig_shape, bias):
    y = np.concatenate([r["yt"].T.astype(np.float32) for r in results], axis=0)
    return y.reshape(orig_shape)


def _sample_ok(y, x2d, kern, bias_flat):
    """Exact numpy check on a 512-token sample (~30 MFLOP)."""
    rows = np.arange(0, TOKENS_TOTAL, TOKENS_TOTAL // 512)[:512]
    xb = x2d[rows].reshape(len(rows), NUM_HEADS, BLOCK_DIM, BLOCK_SIZE)
    yref = (
        np.einsum("nhbs,hbst->nhbt", xb, np.asarray(kern, np.float32)).reshape(
            len(rows), FEATURES
        )
        + bias_flat
    )
    ys = y.reshape(TOKENS_TOTAL, FEATURES)[rows]
    err = np.linalg.norm(ys - yref) / max(float(np.linalg.norm(yref)), 1e-30)
    return err < 5e-3


def kernel(**inputs):
    """Full inputs in, full output out. Shards tokens across 8 cores."""
    global _LAST_EXEC_NS
    import os

    from concourse.bass_utils import run_bass_kernel_spmd

    x = np.asarray(inputs["x"], dtype=np.float32)
    orig_shape = x.shape
    kern = np.asarray(inputs["kernel"], dtype=np.float32)
    bias_flat = np.asarray(inputs["bias"], dtype=np.float32).reshape(FEATURES)

    in_maps = make_in_maps(x, kern, bias_flat)

    if "nc" not in _NC_CACHE:
        _NC_CACHE["nc"] = build_nc()
    nc = _NC_CACHE["nc"]

    trace = bool(os.environ.get("BASS_KERNEL_TRACE"))

    def _canonical():
        # honor trace requests where the NTFF hook exists, but degrade to an
        # untraced run if tracing is unavailable (missing antenv.axon_hooks).
        # Transient device errors (NRT_EXEC_UNIT_UNRECOVERABLE) do NOT clear
        # on immediate retry — escalate: wait for the runtime to reset, then
        # rebuild the PJRT client before the final attempt.
        global _LAST_EXEC_NS
        import time as _time

        attempts = ([(True, 0, False)] if trace else []) + [
            (False, 0, False),
            (False, 10, False),
            (False, 10, True),
        ]
        last_err = None
        for attempt_trace, delay, reset in attempts:
            if delay:
                _time.sleep(delay)
            if reset:
                try:
                    import jax

                    jax.clear_backends()
                    _NC_CACHE.pop("runner", None)  # device buffers now stale
                except Exception:
                    pass
            try:
                res = run_bass_kernel_spmd(
                    nc, in_maps, list(range(N_CORES)), trace=attempt_trace
                )
                _LAST_EXEC_NS = res.exec_time_ns
                return res.results
            except Exception as e:
                last_err = e
        raise last_err

    # first call (and any traced call) uses the canonical library path;
    # repeat calls reuse one cached executable to keep wall time and RSS flat
    results = None
    if _NC_CACHE.get("calls", 0) > 0 and not trace:
        try:
            results = _run_cached(nc, in_maps)
        except Exception:
            results = None
    if results is None:
        results = _canonical()
    _NC_CACHE["calls"] = _NC_CACHE.get("calls", 0) + 1

    x2d = x.reshape(TOKENS_TOTAL, FEATURES)
    y = _assemble(results, orig_shape, bias_flat)
    if not _sample_ok(y, x2d, kern, bias_flat):
        # defense in depth: recompute via the canonical path if the sampled
        # numeric check fails for any reason
        results = _canonical()
        y = _assemble(results, orig_shape, bias_flat)
    return y.reshape(orig_shape)


# revision 8
# speedup vs baseline: 1.0208x; 1.0201x over previous
"""Block-diagonal dense (nn_BlockDiagonalDense) Trainium2 Bass kernel.

Math: x [B=4, T=4096, F=4096] fp32; per token, features are grouped into
512 blocks of 8; each block is multiplied by its own 8x8 matrix
(kernel [16 heads, 32 blocks, 8, 8]) and bias added.

Strategy (v2, fp16 weight-stationary, transposed I/O):
  - Data-parallel over tokens across 8 cores (16384 tokens -> 2048/core).
  - The op is memory-bound (1 GFLOP vs 512 MiB fp32 traffic), and the
    rel-err budget (2e-2) dwarfs fp16 rounding (~1e-3), so all device I/O
    is fp16: half the HBM bytes of the fp32 baseline.
  - Host pre-transposes each core's token slice to x^T [4096 feat, 2048 tok]
    fp16. This removes the on-chip PE transpose entirely: features land on
    partitions, so the per-128-feature-chunk block-diagonal matmul runs
    weight-stationary (lhsT = W chunk [fin,fout], rhs = x^T chunk moving,
    out = y^T [fout, tok] in PSUM).
  - Weights are expanded host-side into 32 chunks of 128x128 block-diagonal
    matrices (fp16), all resident in SBUF; bias folded into the PSUM->SBUF
    drain (DVE tensor_scalar_add on one half, ACT Identity-add on the other,
    so neither engine becomes the bottleneck).
  - Device writes y^T fp16 contiguously; host transposes back and upcasts.
  - Per chunk: 512 KiB in (SP ring), 4 matmuls ap=512, 2 drains, 512 KiB
    out (ACT ring). 32 chunks/core, fully pipelined via tile pools.
"""

import sys

if "/opt/trn_rl_repo" not in sys.path:
    sys.path.insert(0, "/opt/trn_rl_repo")

import numpy as np

NUM_HEADS = 16
BLOCK_SIZE = 8
FEATURES = 4096
HEAD_DIM = FEATURES // NUM_HEADS  # 256
BLOCK_DIM = HEAD_DIM // BLOCK_SIZE  # 32

N_CORES = 8
TOKENS_TOTAL = 4 * 4096  # 16384
TOK_PER_CORE = TOKENS_TOTAL // N_CORES  # 2048

P = 128  # partitions
N_CHUNKS = FEATURES // P  # 32 chunks of 128 features

_NC_CACHE = {}


def build_nc(
    tok_per_core=TOK_PER_CORE,
    repeats=1,
    dma_pattern="split",
    drain_split=1024,
    xbufs=6,
    ybufs=6,
    psbufs=2,
):
    """Build the Bass program for one core processing x^T [4096, tok_per_core].

    repeats>1 wraps the whole body in a hardware loop doing identical work
    (same inputs, same outputs) -- used only for slope-based device timing.

    dma_pattern: "split" = x^T in on SP ring / y^T out on ACT ring;
                 "alt2"  = both rings alternate directions per chunk;
                 "alt3"  = SP + ACT + SWDGE(gpsimd) rotate.
    """
    import contextlib

    import concourse.mybir as mybir
    from concourse import bacc
    from concourse.tile import TileContext

    f32 = mybir.dt.float32
    f16 = mybir.dt.float16
    nc = bacc.Bacc(None, target_bir_lowering=False)

    T = tok_per_core
    xt = nc.declare_dram_parameter("xt", [FEATURES, T], f16, isOutput=False)
    # w: [128 (fi within chunk), 32*128 (chunk-major, fo within chunk)]
    w = nc.declare_dram_parameter("w", [P, N_CHUNKS * P], f16, isOutput=False)
    # b: [128 (fo within chunk), 32 (chunk)]
    b = nc.declare_dram_parameter("b", [P, N_CHUNKS], f32, isOutput=False)
    yt = nc.declare_dram_parameter("yt", [FEATURES, T], f16, isOutput=True)

    TS = T // 4  # one PSUM bank's worth of tokens (512 fp32)

    with TileContext(nc) as tc:
        with (
            tc.tile_pool(name="const", bufs=1) as const_pool,
            tc.tile_pool(name="xin", bufs=xbufs) as x_pool,
            tc.tile_pool(name="yout", bufs=ybufs) as y_pool,
            tc.tile_pool(name="ps", bufs=psbufs, space="PSUM") as ps_pool,
        ):
            w_sb = const_pool.tile([P, N_CHUNKS * P], f16)
            nc.scalar.dma_start(out=w_sb, in_=w[:, :])
            b_sb = const_pool.tile([P, N_CHUNKS], f32)
            nc.scalar.dma_start(out=b_sb, in_=b[:, :])

            if dma_pattern == "split":
                in_engines, out_engines = (nc.sync,), (nc.scalar,)
            elif dma_pattern == "alt2":
                in_engines, out_engines = (nc.sync, nc.scalar), (nc.scalar, nc.sync)
            elif dma_pattern == "alt3":
                in_engines = (nc.sync, nc.gpsimd, nc.scalar)
                out_engines = (nc.scalar, nc.sync, nc.gpsimd)
            else:
                raise ValueError(dma_pattern)

            rep_ctx = (
                tc.For_i(0, repeats, 1) if repeats > 1 else contextlib.nullcontext()
            )
            with rep_ctx:
                for c in range(N_CHUNKS):
                    rows = slice(c * P, (c + 1) * P)
                    xt_sb = x_pool.tile([P, T], f16)
                    in_engines[c % len(in_engines)].dma_start(
                        out=xt_sb, in_=xt[rows, :]
                    )

                    ps = ps_pool.tile([P, T], f32)
                    for t in range(4):
                        nc.tensor.matmul(
                            ps[:, t * TS : (t + 1) * TS],
                            w_sb[:, c * P : (c + 1) * P],
                            xt_sb[:, t * TS : (t + 1) * TS],
                        )

                    y_sb = y_pool.tile([P, T], f16)
                    bc = b_sb[:, c : c + 1]
                    # split the drain across DVE and ACT so neither is the
                    # bottleneck; both fold in the bias add + fp32->fp16 cast
                    nc.vector.tensor_scalar_add(
                        y_sb[:, :drain_split], ps[:, :drain_split], bc
                    )
                    nc.scalar.add(y_sb[:, drain_split:], ps[:, drain_split:], bc)

                    out_engines[c % len(out_engines)].dma_start(
                        out=yt[rows, :], in_=y_sb
                    )

    nc.finalize()
    return nc


def expand_weights(kern):
    """kernel [16, 32, 8, 8] -> [128, 32*128] chunk-major block-diagonal."""
    kern = np.asarray(kern, dtype=np.float32)
    wd = np.zeros((N_CHUNKS, P, P), dtype=np.float32)
    for c in range(N_CHUNKS):
        h = c // 2
        for j in range(16):
            bd = 16 * (c % 2) + j
            wd[c, 8 * j : 8 * j + 8, 8 * j : 8 * j + 8] = kern[h, bd]
    # [chunk, fi, fo] -> [fi, chunk*128 + fo]
    return np.ascontiguousarray(wd.transpose(1, 0, 2).reshape(P, N_CHUNKS * P))


def make_in_maps(x, kern, bias):
    """Host-side prep: shard + transpose + downcast. Returns per-core dicts."""
    xf = np.asarray(x, dtype=np.float32).reshape(TOKENS_TOTAL, FEATURES)
    w16 = expand_weights(kern).astype(np.float16)
    bmat = np.ascontiguousarray(
        np.asarray(bias, dtype=np.float32).reshape(N_CHUNKS, P).T
    )
    maps = []
    for c in range(N_CORES):
        sl = xf[c * TOK_PER_CORE : (c + 1) * TOK_PER_CORE]
        xt_c = np.ascontiguousarray(sl.astype(np.float16).T)
        maps.append({"xt": xt_c, "w": w16, "b": bmat})
    return maps


def reference_numpy(x, kern, bias):
    xb = np.asarray(x, np.float32).reshape(-1, NUM_HEADS, BLOCK_DIM, BLOCK_SIZE)
    k = np.asarray(kern, np.float32)
    y = np.einsum("nhbs,hbst->nhbt", xb, k) + np.asarray(bias, np.float32)
    return y.reshape(x.shape)


_LAST_EXEC_NS = None


def _build_runner(nc):
    """One-time compiled executor for repeat calls: a single cached jit
    (vs run_bass_via_pjrt's per-call closure, which re-compiles and leaks
    ~0.4GB RSS per call) plus device-resident output-zero buffers."""
    import jax
    from jax.experimental.shard_map import shard_map
    from jax.sharding import Mesh, NamedSharding, PartitionSpec

    import concourse.mybir as mybir
    from concourse import bass2jax

    bass2jax.install_neuronx_cc_hook()
    partition_name = nc.partition_id_tensor.name if nc.partition_id_tensor else None
    in_names, out_names, out_avals, zeros = [], [], [], []
    for alloc in nc.m.functions[0].allocations:
        if not isinstance(alloc, mybir.MemoryLocationSet):
            continue
        name = alloc.memorylocations[0].name
        if alloc.kind == "ExternalInput":
            if name != partition_name:
                in_names.append(name)
        elif alloc.kind == "ExternalOutput":
            shape = tuple(alloc.tensor_shape)
            dt = mybir.dt.np(alloc.dtype)
            out_names.append(name)
            out_avals.append(jax.core.ShapedArray(shape, dt))
            zeros.append(np.zeros(shape, dt))
    all_names = in_names + out_names + ([partition_name] if partition_name else [])

    def _body(*args):
        ops = list(args)
        if partition_name is not None:
            ops.append(bass2jax.partition_id_tensor())
        return tuple(
            bass2jax._bass_exec_p.bind(
                *ops,
                out_avals=tuple(out_avals),
                in_names=tuple(all_names),
                out_names=tuple(out_names),
                lowering_input_output_aliases=(),
                sim_require_finite=True,
                sim_require_nnan=True,
                nc=nc,
            )
        )

    devices = jax.devices()[:N_CORES]
    mesh = Mesh(np.asarray(devices), ("core",))
    n_all = len(in_names) + len(out_names)
    fn = jax.jit(
        shard_map(
            _body,
            mesh=mesh,
            in_specs=(PartitionSpec("core"),) * n_all,
            out_specs=(PartitionSpec("core"),) * len(out_names),
            check_rep=False,
        ),
        keep_unused=True,
    )
    shard = NamedSharding(mesh, PartitionSpec("core"))
    dev_zeros = [
        jax.device_put(np.zeros((N_CORES * z.shape[0], *z.shape[1:]), z.dtype), shard)
        for z in zeros
    ]
    out_shapes = [z.shape for z in zeros]
    return fn, in_names, out_names, out_shapes, dev_zeros, shard


def _run_cached(nc, in_maps):
    import jax

    if "runner" not in _NC_CACHE:
        _NC_CACHE["runner"] = _build_runner(nc)
    fn, in_names, out_names, out_shapes, dev_zeros, shard = _NC_CACHE["runner"]
    concat_in = [
        np.concatenate([np.asarray(m[n]) for m in in_maps], axis=0)
        for n in in_names
    ]
    dev_in = [jax.device_put(a, shard) for a in concat_in]
    outs = fn(*dev_in, *dev_zeros)
    return [
        {
            name: np.asarray(outs[i]).reshape(N_CORES, *out_shapes[i])[c]
            for i, name in enumerate(out_names)
        }
        for c in range(N_CORES)
    ]


def _assemble(results, orig_shape, bias):
    y = np.concatenate([r["yt"].T.astype(np.float32) for r in results], axis=0)
    return y.reshape(orig_shape)


def _sample_ok(y, x2d, kern, bias_flat):
    """Exact numpy check on a 512-token sample (~30 MFLOP)."""
    rows = np.arange(0, TOKENS_TOTAL, TOKENS_TOTAL // 512)[:512]
    xb = x2d[rows].reshape(len(rows), NUM_HEADS, BLOCK_DIM, BLOCK_SIZE)
    yref = (
        np.einsum("nhbs,hbst->nhbt", xb, np.asarray(kern, np.float32)).reshape(
            len(rows), FEATURES
        )
        + bias_flat
    )
    ys = y.reshape(TOKENS_TOTAL, FEATURES)[rows]
    err = np.linalg.norm(ys - yref) / max(float(np.linalg.norm(yref)), 1e-30)
    return err < 5e-3


def kernel(**inputs):
    """Full inputs in, full output out. Shards tokens across 8 cores."""
    global _LAST_EXEC_NS
    import os

    from concourse.bass_utils import run_bass_kernel_spmd

    x = np.asarray(inputs["x"], dtype=np.float32)
    orig_shape = x.shape
    kern = np.asarray(inputs["kernel"], dtype=np.float32)
    bias_flat = np.asarray(inputs["bias"], dtype=np.float32).reshape(FEATURES)

    in_maps = make_in_maps(x, kern, bias_flat)

    if "nc" not in _NC_CACHE:
        _NC_CACHE["nc"] = build_nc()
    nc = _NC_CACHE["nc"]

    trace = bool(os.environ.get("BASS_KERNEL_TRACE"))

    def _canonical():
        # honor trace requests where the NTFF hook exists, but degrade to an
        # untraced run if tracing is unavailable (missing antenv.axon_hooks).
        # Transient device errors (NRT_EXEC_UNIT_UNRECOVERABLE) do NOT clear
        # on immediate retry — escalate: wait for the runtime to reset, then
        # rebuild the PJRT client before the final attempt.
        global _LAST_EXEC_NS
        import time as _time

        attempts = ([(True, 0, False)] if trace else []) + [
            (False, 0, False),
            (False, 10, False),
            (False, 10, True),
        ]
        last_err = None
        for attempt_trace, delay, reset in attempts:
            if delay:
                _time.sleep(delay)
            if reset:
                try:
                    import jax.extend as _jex

                    _jex.backend.clear_backends()
                    _NC_CACHE.pop("runner", None)  # device buffers now stale
                except Exception:
                    pass
            try:
                res = run_bass_kernel_spmd(
                    nc, in_maps, list(range(N_CORES)), trace=attempt_trace
                )
                _LAST_EXEC_NS = res.exec_time_ns
                return res.results
            except Exception as e:
                last_err = e
        raise last_err

    # first call (and any traced call) uses the canonical library path;
    # repeat calls reuse one cached executable to keep wall time and RSS flat
    results = None
    if _NC_CACHE.get("calls", 0) > 0 and not trace:
        try:
            results = _run_cached(nc, in_maps)
        except Exception:
            results = None
    if results is None:
        results = _canonical()
    _NC_CACHE["calls"] = _NC_CACHE.get("calls", 0) + 1

    x2d = x.reshape(TOKENS_TOTAL, FEATURES)
    y = _assemble(results, orig_shape, bias_flat)
    if not _sample_ok(y, x2d, kern, bias_flat):
        # defense in depth: recompute via the canonical path if the sampled
        # numeric check fails for any reason
        results = _canonical()
        y = _assemble(results, orig_shape, bias_flat)
    return y.reshape(orig_shape)
